# revision 13
# baseline (speedup 1.0000x reference)
"""Stereo cost volume on 8 Trainium2 NeuronCores (batch-parallel SPMD).

out[b,h,w,d] = sum_c ref[b,h,w+63-d,c] * aux[b,h,w,c]
  B=8, H=192, W=384, C=128, D=64, ref width 447.

Strategy:
  * Shard batch across the 8 cores (1 batch each); pure SPMD, no collectives.
  * Host pre-transposes inputs to [C, H, W] fp16 so the channel contraction
    (C=128) lands on SBUF partitions and feeds the 128x128 PE array exactly.
  * Per h-row, per 128-wide W chunk: 4 col-tiled matmuls (M=32 output
    positions each, tile_position=(0,32g)) stream a 95-column ref window into
    one PSUM bank laid out [128, 288].  Grouping output w-positions by 32
    bounds each group's diagonal band inside 95 uniform columns, so no
    per-partition (diagonal) addressing is ever needed on device.
  * One DVE copy PSUM->SBUF per h, large contiguous DMAs in/out.
  * Host extracts the diagonal band from the [128, H, 288] per-core output
    with a zero-copy as_strided view (shear is free on the host).
"""

import sys

import numpy as np

sys.path.insert(0, "/opt/trn_rl_repo")

import concourse.bass as bass
import concourse.mybir as mybir
from concourse import bacc, bass_utils
from concourse.tile import TileContext

B, H, W, C, D = 8, 192, 384, 128, 64
OFF = 63
REF_W = W + OFF  # 447
NCHUNK = W // 128  # 3
GW = 32  # output w-positions per col group
NGROUP = 128 // GW  # 4
WIN = GW + OFF  # 95 streamed ref columns per group
BLK = 96  # column stride per chunk block in PSUM
SHIP = 80  # shipped columns per chunk block (16-row subgroup window)
OUT_COLS = NCHUNK * SHIP  # 240
HB = 8  # h rows per DMA block

F16 = mybir.dt.float16
F32 = mybir.dt.float32


def _build() -> bass.Bass:
    nc = bacc.Bacc("TRN2", target_bir_lowering=False, debug=False)
    ref_d = nc.dram_tensor("ref_t", [C, H, REF_W], F16, kind="ExternalInput").ap()
    aux_d = nc.dram_tensor("aux_t", [C, H, W], F16, kind="ExternalInput").ap()
    # [parity, gg, rr, h, 3*80]: device row m = 32*gg + 16*parity + rr ships the
    # 80-col window starting at col 16*parity of each 96-wide chunk block.
    out_d = nc.dram_tensor(
        "out_raw", [2, 4, 16, H, OUT_COLS], F32, kind="ExternalOutput"
    ).ap()

    with TileContext(nc) as tc:
        with (
            tc.tile_pool(name="inp", bufs=3) as inp,
            tc.tile_pool(name="outp", bufs=3) as outp,
            tc.tile_pool(name="ps", bufs=6, space="PSUM") as ps,
        ):
            for hb in range(0, H, HB):
                ref_sb = inp.tile([C, HB * REF_W], F16, tag="ref")
                aux_sb = inp.tile([C, HB * W], F16, tag="aux")
                nc.sync.dma_start(out=ref_sb, in_=ref_d[:, hb : hb + HB, :])
                nc.sync.dma_start(out=aux_sb, in_=aux_d[:, hb : hb + HB, :])
                even_sb = outp.tile([128, HB * OUT_COLS], F32, tag="even")
                odd_sb = outp.tile([128, HB * OUT_COLS], F32, tag="odd")
                for hl in range(HB):
                    pt = ps.tile([128, NCHUNK * BLK], F32)
                    for k in range(NCHUNK):
                        for g in range(NGROUP):
                            w0 = 128 * k + GW * g
                            nc.tensor.matmul(
                                out=pt[GW * g : GW * g + GW, BLK * k : BLK * k + WIN],
                                lhsT=aux_sb[:, hl * W + w0 : hl * W + w0 + GW],
                                rhs=ref_sb[:, hl * REF_W + w0 : hl * REF_W + w0 + WIN],
                                start=True,
                                stop=True,
                                tile_position=(0, GW * g),
                            )
                    pt3 = pt.rearrange("p (k c) -> p k c", k=NCHUNK)
                    # rows 16*parity..+16 of every 32-group need cols
                    # [16*parity, 16*parity+80) of each 96-block; copy both
                    # windows for all rows (DVE + ACT share the work) and ship
                    # each parity's 16-row slices from its own buffer.
                    nc.vector.tensor_copy(
                        out=even_sb[:, hl * OUT_COLS : (hl + 1) * OUT_COLS],
                        in_=pt3[:, :, 0:SHIP],
                    )
                    nc.scalar.copy(
                        out=odd_sb[:, hl * OUT_COLS : (hl + 1) * OUT_COLS],
                        in_=pt3[:, :, 16 : 16 + SHIP],
                    )
                # outputs go out on the Activation HWDGE queue so they don't
                # serialize behind input loads on the sync queue
                for sub in range(8):
                    par, gg = sub & 1, sub >> 1
                    src = odd_sb if par else even_sb
                    nc.scalar.dma_start(
                        out=out_d[par, gg, :, hb : hb + HB, :],
                        in_=src[16 * sub : 16 * sub + 16, :],
                    )
    nc.compile()
    return nc


def _extract(core_out: np.ndarray) -> np.ndarray:
    """[2, 4, 16, H, 240] f32 device output -> [H, W, D] cost volume (one batch).

    Device row m = 32*gg + 16*parity + rr ships cost(w=128k+m, ref=128k+32gg+
    16*parity+c') at [parity, gg, rr, h, 80k + c' - 16*parity]; the band entry
    for disparity d sits at shipped col 80k + rr + 63 - d.
    """
    sq, sg, sr, sh, sc = core_out.strides
    base = core_out[:, :, :, :, OFF:]
    v = np.lib.stride_tricks.as_strided(
        base,
        shape=(H, NCHUNK, 4, 2, 16, D),
        strides=(sh, SHIP * sc, sg, sq, sr + sc, -sc),
    )
    return v.reshape(H, W, D)


LAST_RESULTS = None


def kernel(ref: np.ndarray, aux: np.ndarray, _trace: bool = False) -> np.ndarray:
    global LAST_RESULTS
    ref16 = np.ascontiguousarray(ref.astype(np.float16).transpose(0, 3, 1, 2))
    aux16 = np.ascontiguousarray(aux.astype(np.float16).transpose(0, 3, 1, 2))
    nc = _build()
    in_maps = [{"ref_t": ref16[b], "aux_t": aux16[b]} for b in range(B)]
    res = bass_utils.run_bass_kernel_spmd(nc, in_maps, list(range(B)), trace=_trace)
    LAST_RESULTS = res
    return np.stack([_extract(res.results[b]["out_raw"]) for b in range(B)], axis=0)


# revision 14
# speedup vs baseline: 1.1223x; 1.1223x over previous
"""Stereo cost volume on 8 Trainium2 NeuronCores (batch-parallel SPMD).

out[b,h,w,d] = sum_c ref[b,h,w+63-d,c] * aux[b,h,w,c]
  B=8, H=192, W=384, C=128, D=64, ref width 447.

Strategy:
  * Shard batch across the 8 cores (1 batch each); pure SPMD, no collectives.
  * Host pre-transposes inputs to [C, H, W] fp16 so the channel contraction
    (C=128) lands on SBUF partitions and feeds the 128x128 PE array exactly.
  * Per h-row, per 128-wide W chunk: 4 col-tiled matmuls (M=32 output
    positions each, tile_position=(0,32g)) stream a 95-column ref window into
    one PSUM bank laid out [128, 288].  Grouping output w-positions by 32
    bounds each group's diagonal band inside 95 uniform columns, so no
    per-partition (diagonal) addressing is ever needed on device.
  * One DVE copy PSUM->SBUF per h, large contiguous DMAs in/out.
  * Host extracts the diagonal band from the [128, H, 288] per-core output
    with a zero-copy as_strided view (shear is free on the host).
"""

import sys

import numpy as np

sys.path.insert(0, "/opt/trn_rl_repo")

import concourse.bass as bass
import concourse.mybir as mybir
from concourse import bacc, bass_utils
from concourse.tile import TileContext

B, H, W, C, D = 8, 192, 384, 128, 64
OFF = 63
REF_W = W + OFF  # 447
NCHUNK = W // 128  # 3
GW = 32  # output w-positions per col group
NGROUP = 128 // GW  # 4
WIN = GW + OFF  # 95 streamed ref columns per group
BLK = 96  # column stride per chunk block in PSUM
SHIP = 80  # shipped columns per chunk block (16-row subgroup window)
OUT_COLS = NCHUNK * SHIP  # 240
HB = 16  # h rows per DMA block

F16 = mybir.dt.float16
F32 = mybir.dt.float32


def _build() -> bass.Bass:
    nc = bacc.Bacc("TRN2", target_bir_lowering=False, debug=False)
    ref_d = nc.dram_tensor("ref_t", [C, H, REF_W], F16, kind="ExternalInput").ap()
    aux_d = nc.dram_tensor("aux_t", [C, H, W], F16, kind="ExternalInput").ap()
    # [parity, gg, rr, h, 3*80]: device row m = 32*gg + 16*parity + rr ships the
    # 80-col window starting at col 16*parity of each 96-wide chunk block.
    out_d = nc.dram_tensor(
        "out_raw", [2, 4, 16, H, OUT_COLS], F32, kind="ExternalOutput"
    ).ap()

    with TileContext(nc) as tc:
        with (
            tc.tile_pool(name="inp", bufs=3) as inp,
            tc.tile_pool(name="outp", bufs=3) as outp,
            tc.tile_pool(name="ps", bufs=6, space="PSUM") as ps,
        ):
            for hb in range(0, H, HB):
                ref_sb = inp.tile([C, HB * REF_W], F16, tag="ref")
                aux_sb = inp.tile([C, HB * W], F16, tag="aux")
                nc.sync.dma_start(out=ref_sb, in_=ref_d[:, hb : hb + HB, :])
                nc.sync.dma_start(out=aux_sb, in_=aux_d[:, hb : hb + HB, :])
                even_sb = outp.tile([128, HB * OUT_COLS], F32, tag="even")
                odd_sb = outp.tile([128, HB * OUT_COLS], F32, tag="odd")
                for hl in range(HB):
                    pt = ps.tile([128, NCHUNK * BLK], F32)
                    for k in range(NCHUNK):
                        for g in range(NGROUP):
                            w0 = 128 * k + GW * g
                            nc.tensor.matmul(
                                out=pt[GW * g : GW * g + GW, BLK * k : BLK * k + WIN],
                                lhsT=aux_sb[:, hl * W + w0 : hl * W + w0 + GW],
                                rhs=ref_sb[:, hl * REF_W + w0 : hl * REF_W + w0 + WIN],
                                start=True,
                                stop=True,
                                tile_position=(0, GW * g),
                            )
                    pt3 = pt.rearrange("p (k c) -> p k c", k=NCHUNK)
                    # rows 16*parity..+16 of every 32-group need cols
                    # [16*parity, 16*parity+80) of each 96-block; copy both
                    # windows for all rows (DVE + ACT share the work) and ship
                    # each parity's 16-row slices from its own buffer.
                    nc.vector.tensor_copy(
                        out=even_sb[:, hl * OUT_COLS : (hl + 1) * OUT_COLS],
                        in_=pt3[:, :, 0:SHIP],
                    )
                    nc.scalar.copy(
                        out=odd_sb[:, hl * OUT_COLS : (hl + 1) * OUT_COLS],
                        in_=pt3[:, :, 16 : 16 + SHIP],
                    )
                # outputs go out on the Activation HWDGE queue so they don't
                # serialize behind input loads on the sync queue
                for sub in range(8):
                    par, gg = sub & 1, sub >> 1
                    src = odd_sb if par else even_sb
                    nc.scalar.dma_start(
                        out=out_d[par, gg, :, hb : hb + HB, :],
                        in_=src[16 * sub : 16 * sub + 16, :],
                    )
    nc.compile()
    return nc


def _extract(core_out: np.ndarray) -> np.ndarray:
    """[2, 4, 16, H, 240] f32 device output -> [H, W, D] cost volume (one batch).

    Device row m = 32*gg + 16*parity + rr ships cost(w=128k+m, ref=128k+32gg+
    16*parity+c') at [parity, gg, rr, h, 80k + c' - 16*parity]; the band entry
    for disparity d sits at shipped col 80k + rr + 63 - d.
    """
    sq, sg, sr, sh, sc = core_out.strides
    base = core_out[:, :, :, :, OFF:]
    v = np.lib.stride_tricks.as_strided(
        base,
        shape=(H, NCHUNK, 4, 2, 16, D),
        strides=(sh, SHIP * sc, sg, sq, sr + sc, -sc),
    )
    return v.reshape(H, W, D)


LAST_RESULTS = None


def kernel(ref: np.ndarray, aux: np.ndarray, _trace: bool = False) -> np.ndarray:
    global LAST_RESULTS
    ref16 = np.ascontiguousarray(ref.astype(np.float16).transpose(0, 3, 1, 2))
    aux16 = np.ascontiguousarray(aux.astype(np.float16).transpose(0, 3, 1, 2))
    nc = _build()
    in_maps = [{"ref_t": ref16[b], "aux_t": aux16[b]} for b in range(B)]
    res = bass_utils.run_bass_kernel_spmd(nc, in_maps, list(range(B)), trace=_trace)
    LAST_RESULTS = res
    return np.stack([_extract(res.results[b]["out_raw"]) for b in range(B)], axis=0)


# revision 18
# speedup vs baseline: 1.3157x; 1.1723x over previous
"""Stereo cost volume on 8 Trainium2 NeuronCores (batch-parallel SPMD).

out[b,h,w,d] = sum_c ref[b,h,w+63-d,c] * aux[b,h,w,c]
  B=8, H=192, W=384, C=128, D=64, ref width 447.

Strategy:
  * Shard batch across the 8 cores (1 batch each); pure SPMD, no collectives.
  * Host pre-transposes inputs to [C, H, W] fp16 so the channel contraction
    (C=128) lands on SBUF partitions and feeds the 128x128 PE array exactly.
  * Per h-row, per 128-wide W chunk: 4 col-tiled matmuls (M=32 output
    positions each, tile_position=(0,32g)) stream a 95-column ref window into
    one PSUM bank laid out [128, 288].  Grouping output w-positions by 32
    bounds each group's diagonal band inside 95 uniform columns, so no
    per-partition (diagonal) addressing is ever needed on device.
  * One DVE copy PSUM->SBUF per h, large contiguous DMAs in/out.
  * Host extracts the diagonal band from the [128, H, 288] per-core output
    with a zero-copy as_strided view (shear is free on the host).
"""

import sys

import numpy as np

sys.path.insert(0, "/opt/trn_rl_repo")

import concourse.bass as bass
import concourse.mybir as mybir
from concourse import bacc, bass_utils
from concourse.tile import TileContext

B, H, W, C, D = 8, 192, 384, 128, 64
OFF = 63
REF_W = W + OFF  # 447
NCHUNK = W // 128  # 3
GW = 32  # output w-positions per col group
NGROUP = 128 // GW  # 4
WIN = GW + OFF  # 95 streamed ref columns per group
BLK = 96  # column stride per chunk block in PSUM
OUT_COLS = NCHUNK * BLK  # 288
HB = 16  # h rows per DMA block

F16 = mybir.dt.float16
F32 = mybir.dt.float32


def _build() -> bass.Bass:
    nc = bacc.Bacc("TRN2", target_bir_lowering=False, debug=False)
    ref_d = nc.dram_tensor("ref_t", [C, H, REF_W], F16, kind="ExternalInput").ap()
    aux_d = nc.dram_tensor("aux_t", [C, H, W], F16, kind="ExternalInput").ap()
    out_d = nc.dram_tensor("out_raw", [128, H, OUT_COLS], F32, kind="ExternalOutput").ap()

    with TileContext(nc) as tc:
        with (
            tc.tile_pool(name="inp", bufs=3) as inp,
            tc.tile_pool(name="outp", bufs=3) as outp,
            tc.tile_pool(name="ps", bufs=6, space="PSUM") as ps,
        ):
            for hb in range(0, H, HB):
                ref_sb = inp.tile([C, HB * REF_W], F16, tag="ref")
                aux_sb = inp.tile([C, HB * W], F16, tag="aux")
                nc.sync.dma_start(out=ref_sb, in_=ref_d[:, hb : hb + HB, :])
                nc.sync.dma_start(out=aux_sb, in_=aux_d[:, hb : hb + HB, :])
                out_sb = outp.tile([128, HB * OUT_COLS], F32, tag="out")
                for hl in range(HB):
                    pt = ps.tile([128, NCHUNK * BLK], F32)
                    for k in range(NCHUNK):
                        for g in range(NGROUP):
                            w0 = 128 * k + GW * g
                            nc.tensor.matmul(
                                out=pt[GW * g : GW * g + GW, BLK * k : BLK * k + WIN],
                                lhsT=aux_sb[:, hl * W + w0 : hl * W + w0 + GW],
                                rhs=ref_sb[:, hl * REF_W + w0 : hl * REF_W + w0 + WIN],
                                start=True,
                                stop=True,
                                tile_position=(0, GW * g),
                            )
                    nc.vector.tensor_copy(
                        out=out_sb[:, hl * OUT_COLS : (hl + 1) * OUT_COLS], in_=pt
                    )
                # outputs go out on the Activation HWDGE queue so they don't
                # serialize behind input loads on the sync queue
                nc.scalar.dma_start(out=out_d[:, hb : hb + HB, :], in_=out_sb)
    nc.compile()
    return nc


def _extract(core_out: np.ndarray) -> np.ndarray:
    """[128, H, 288] f32 device output -> [H, W, D] cost volume (one batch).

    Device row m = 32g + r, column 96k + c holds
    dot(aux[128k + m], ref[128k + 32g + c]); the band entry for
    w = 128k + m, disparity d sits at c = r + 63 - d.
    """
    sm, sh, sc = core_out.strides
    base = core_out[:, :, OFF:]
    v = np.lib.stride_tricks.as_strided(
        base,
        shape=(H, NCHUNK, NGROUP, GW, D),
        strides=(sh, BLK * sc, GW * sm, sm + sc, -sc),
    )
    return v.reshape(H, W, D)


LAST_RESULTS = None


def kernel(ref: np.ndarray, aux: np.ndarray, _trace: bool = False) -> np.ndarray:
    global LAST_RESULTS
    ref16 = np.ascontiguousarray(ref.astype(np.float16).transpose(0, 3, 1, 2))
    aux16 = np.ascontiguousarray(aux.astype(np.float16).transpose(0, 3, 1, 2))
    nc = _build()
    in_maps = [{"ref_t": ref16[b], "aux_t": aux16[b]} for b in range(B)]
    res = bass_utils.run_bass_kernel_spmd(nc, in_maps, list(range(B)), trace=_trace)
    LAST_RESULTS = res
    return np.stack([_extract(res.results[b]["out_raw"]) for b in range(B)], axis=0)


# revision 20
# speedup vs baseline: 1.3499x; 1.0260x over previous
"""Stereo cost volume on 8 Trainium2 NeuronCores (batch-parallel SPMD).

out[b,h,w,d] = sum_c ref[b,h,w+63-d,c] * aux[b,h,w,c]
  B=8, H=192, W=384, C=128, D=64, ref width 447.

Strategy:
  * Shard batch across the 8 cores (1 batch each); pure SPMD, no collectives.
  * Host pre-transposes inputs to [C, H, W] fp16 so the channel contraction
    (C=128) lands on SBUF partitions and feeds the 128x128 PE array exactly.
  * Per h-row, per 128-wide W chunk: 4 col-tiled matmuls (M=32 output
    positions each, tile_position=(0,32g)) stream a 95-column ref window into
    one PSUM bank laid out [128, 288].  Grouping output w-positions by 32
    bounds each group's diagonal band inside 95 uniform columns, so no
    per-partition (diagonal) addressing is ever needed on device.
  * One DVE copy PSUM->SBUF per h, large contiguous DMAs in/out.
  * Host extracts the diagonal band from the [128, H, 288] per-core output
    with a zero-copy as_strided view (shear is free on the host).
"""

import sys

import numpy as np

sys.path.insert(0, "/opt/trn_rl_repo")

import concourse.bass as bass
import concourse.mybir as mybir
from concourse import bacc, bass_utils
from concourse.tile import TileContext

B, H, W, C, D = 8, 192, 384, 128, 64
OFF = 63
REF_W = W + OFF  # 447
NCHUNK = W // 128  # 3
GW = 32  # output w-positions per col group
NGROUP = 128 // GW  # 4
WIN = GW + OFF  # 95 streamed ref columns per group
BLK = 96  # column stride per chunk block in PSUM
OUT_COLS = NCHUNK * BLK  # 288
HB = 16  # h rows per DMA block

F16 = mybir.dt.float16
F32 = mybir.dt.float32


def _build() -> bass.Bass:
    nc = bacc.Bacc("TRN2", target_bir_lowering=False, debug=False)
    ref_d = nc.dram_tensor("ref_t", [C, H, REF_W], F16, kind="ExternalInput").ap()
    aux_d = nc.dram_tensor("aux_t", [C, H, W], F16, kind="ExternalInput").ap()
    out_d = nc.dram_tensor("out_raw", [128, H, OUT_COLS], F32, kind="ExternalOutput").ap()

    with TileContext(nc) as tc:
        with (
            tc.tile_pool(name="inp", bufs=3) as inp,
            tc.tile_pool(name="outp", bufs=3) as outp,
            tc.tile_pool(name="ps", bufs=6, space="PSUM") as ps,
        ):
            # taper block sizes: small first block gets the pipeline rolling
            # sooner; small last block shrinks the compute+store drain tail
            blocks = [4, 8] + [16] * 11 + [4]
            assert sum(blocks) == H
            hb = 0
            for nh in blocks:
                ref_sb = inp.tile([C, HB * REF_W], F16, tag="ref")
                aux_sb = inp.tile([C, HB * W], F16, tag="aux")
                nc.sync.dma_start(out=ref_sb[:, : nh * REF_W], in_=ref_d[:, hb : hb + nh, :])
                nc.sync.dma_start(out=aux_sb[:, : nh * W], in_=aux_d[:, hb : hb + nh, :])
                out_sb = outp.tile([128, HB * OUT_COLS], F32, tag="out")
                for hl in range(nh):
                    pt = ps.tile([128, NCHUNK * BLK], F32)
                    for k in range(NCHUNK):
                        for g in range(NGROUP):
                            w0 = 128 * k + GW * g
                            nc.tensor.matmul(
                                out=pt[GW * g : GW * g + GW, BLK * k : BLK * k + WIN],
                                lhsT=aux_sb[:, hl * W + w0 : hl * W + w0 + GW],
                                rhs=ref_sb[:, hl * REF_W + w0 : hl * REF_W + w0 + WIN],
                                start=True,
                                stop=True,
                                tile_position=(0, GW * g),
                            )
                    nc.vector.tensor_copy(
                        out=out_sb[:, hl * OUT_COLS : (hl + 1) * OUT_COLS], in_=pt
                    )
                # outputs go out on the Activation HWDGE queue so they don't
                # serialize behind input loads on the sync queue; half-block
                # granularity lets stores start before the block finishes
                for h0 in range(0, nh, 8):
                    h1 = min(h0 + 8, nh)
                    nc.scalar.dma_start(
                        out=out_d[:, hb + h0 : hb + h1, :],
                        in_=out_sb[:, h0 * OUT_COLS : h1 * OUT_COLS],
                    )
                hb += nh
    nc.compile()
    return nc


def _extract(core_out: np.ndarray) -> np.ndarray:
    """[128, H, 288] f32 device output -> [H, W, D] cost volume (one batch).

    Device row m = 32g + r, column 96k + c holds
    dot(aux[128k + m], ref[128k + 32g + c]); the band entry for
    w = 128k + m, disparity d sits at c = r + 63 - d.
    """
    sm, sh, sc = core_out.strides
    base = core_out[:, :, OFF:]
    v = np.lib.stride_tricks.as_strided(
        base,
        shape=(H, NCHUNK, NGROUP, GW, D),
        strides=(sh, BLK * sc, GW * sm, sm + sc, -sc),
    )
    return v.reshape(H, W, D)


LAST_RESULTS = None


def kernel(ref: np.ndarray, aux: np.ndarray, _trace: bool = False) -> np.ndarray:
    global LAST_RESULTS
    ref16 = np.ascontiguousarray(ref.astype(np.float16).transpose(0, 3, 1, 2))
    aux16 = np.ascontiguousarray(aux.astype(np.float16).transpose(0, 3, 1, 2))
    nc = _build()
    in_maps = [{"ref_t": ref16[b], "aux_t": aux16[b]} for b in range(B)]
    res = bass_utils.run_bass_kernel_spmd(nc, in_maps, list(range(B)), trace=_trace)
    LAST_RESULTS = res
    return np.stack([_extract(res.results[b]["out_raw"]) for b in range(B)], axis=0)


# revision 22
# speedup vs baseline: 1.3553x; 1.0039x over previous
"""Stereo cost volume on 8 Trainium2 NeuronCores (batch-parallel SPMD).

out[b,h,w,d] = sum_c ref[b,h,w+63-d,c] * aux[b,h,w,c]
  B=8, H=192, W=384, C=128, D=64, ref width 447.

Strategy:
  * Shard batch across the 8 cores (1 batch each); pure SPMD, no collectives.
  * Host pre-transposes inputs to [C, H, W] fp16 so the channel contraction
    (C=128) lands on SBUF partitions and feeds the 128x128 PE array exactly.
  * Per h-row, per 128-wide W chunk: 4 col-tiled matmuls (M=32 output
    positions each, tile_position=(0,32g)) stream a 95-column ref window into
    one PSUM bank laid out [128, 288].  Grouping output w-positions by 32
    bounds each group's diagonal band inside 95 uniform columns, so no
    per-partition (diagonal) addressing is ever needed on device.
  * One DVE copy PSUM->SBUF per h, large contiguous DMAs in/out.
  * Host extracts the diagonal band from the [128, H, 288] per-core output
    with a zero-copy as_strided view (shear is free on the host).
"""

import sys

import numpy as np

sys.path.insert(0, "/opt/trn_rl_repo")

import concourse.bass as bass
import concourse.mybir as mybir
from concourse import bacc, bass_utils
from concourse.tile import TileContext

B, H, W, C, D = 8, 192, 384, 128, 64
OFF = 63
REF_W = W + OFF  # 447
NCHUNK = W // 128  # 3
GW = 32  # output w-positions per col group
NGROUP = 128 // GW  # 4
WIN = GW + OFF  # 95 streamed ref columns per group
BLK = 96  # column stride per chunk block in PSUM
OUT_COLS = NCHUNK * BLK  # 288
HB = 16  # h rows per DMA block

F16 = mybir.dt.float16
F32 = mybir.dt.float32


def _build() -> bass.Bass:
    nc = bacc.Bacc("TRN2", target_bir_lowering=False, debug=False)
    ref_d = nc.dram_tensor("ref_t", [C, H, REF_W], F16, kind="ExternalInput").ap()
    aux_d = nc.dram_tensor("aux_t", [C, H, W], F16, kind="ExternalInput").ap()
    out_d = nc.dram_tensor("out_raw", [128, H, OUT_COLS], F32, kind="ExternalOutput").ap()

    with TileContext(nc) as tc:
        with (
            tc.tile_pool(name="inp", bufs=3) as inp,
            tc.tile_pool(name="outp", bufs=3) as outp,
            tc.tile_pool(name="ps", bufs=6, space="PSUM") as ps,
        ):
            # taper block sizes: small first block gets the pipeline rolling
            # sooner; small last blocks shrink the compute+store drain tail
            blocks = [4, 8] + [16] * 10 + [8, 4, 4, 2, 2]
            assert sum(blocks) == H
            hb = 0
            for nh in blocks:
                ref_sb = inp.tile([C, HB * REF_W], F16, tag="ref")
                aux_sb = inp.tile([C, HB * W], F16, tag="aux")
                nc.sync.dma_start(out=ref_sb[:, : nh * REF_W], in_=ref_d[:, hb : hb + nh, :])
                nc.sync.dma_start(out=aux_sb[:, : nh * W], in_=aux_d[:, hb : hb + nh, :])
                out_sb = outp.tile([128, HB * OUT_COLS], F32, tag="out")
                for hl in range(nh):
                    pt = ps.tile([128, NCHUNK * BLK], F32)
                    for k in range(NCHUNK):
                        for g in range(NGROUP):
                            w0 = 128 * k + GW * g
                            nc.tensor.matmul(
                                out=pt[GW * g : GW * g + GW, BLK * k : BLK * k + WIN],
                                lhsT=aux_sb[:, hl * W + w0 : hl * W + w0 + GW],
                                rhs=ref_sb[:, hl * REF_W + w0 : hl * REF_W + w0 + WIN],
                                start=True,
                                stop=True,
                                tile_position=(0, GW * g),
                            )
                    # ACT takes every 4th copy so the PSUM-eviction latency
                    # doesn't serialize entirely on DVE near the kernel tail
                    copy_eng = nc.scalar.copy if hl % 4 == 3 else nc.vector.tensor_copy
                    copy_eng(
                        out=out_sb[:, hl * OUT_COLS : (hl + 1) * OUT_COLS], in_=pt
                    )
                # outputs go out on the Activation HWDGE queue so they don't
                # serialize behind input loads on the sync queue; quarter-block
                # granularity lets stores start before the block finishes
                for h0 in range(0, nh, 4):
                    h1 = min(h0 + 4, nh)
                    nc.scalar.dma_start(
                        out=out_d[:, hb + h0 : hb + h1, :],
                        in_=out_sb[:, h0 * OUT_COLS : h1 * OUT_COLS],
                    )
                hb += nh
    nc.compile()
    return nc


def _extract(core_out: np.ndarray) -> np.ndarray:
    """[128, H, 288] f32 device output -> [H, W, D] cost volume (one batch).

    Device row m = 32g + r, column 96k + c holds
    dot(aux[128k + m], ref[128k + 32g + c]); the band entry for
    w = 128k + m, disparity d sits at c = r + 63 - d.
    """
    sm, sh, sc = core_out.strides
    base = core_out[:, :, OFF:]
    v = np.lib.stride_tricks.as_strided(
        base,
        shape=(H, NCHUNK, NGROUP, GW, D),
        strides=(sh, BLK * sc, GW * sm, sm + sc, -sc),
    )
    return v.reshape(H, W, D)


LAST_RESULTS = None


def kernel(ref: np.ndarray, aux: np.ndarray, _trace: bool = False) -> np.ndarray:
    global LAST_RESULTS
    ref16 = np.ascontiguousarray(ref.astype(np.float16).transpose(0, 3, 1, 2))
    aux16 = np.ascontiguousarray(aux.astype(np.float16).transpose(0, 3, 1, 2))
    nc = _build()
    in_maps = [{"ref_t": ref16[b], "aux_t": aux16[b]} for b in range(B)]
    res = bass_utils.run_bass_kernel_spmd(nc, in_maps, list(range(B)), trace=_trace)
    LAST_RESULTS = res
    return np.stack([_extract(res.results[b]["out_raw"]) for b in range(B)], axis=0)
